# revision 10
# baseline (speedup 1.0000x reference)
"""Two-layer GAT (gnn_message_passing) on 8 Trainium2 NeuronCores — v2.

Sharding: nodes split into 8 contiguous shards of 1024 (one per core); each
core owns every edge whose destination is in its shard.  Halo exchange is an
on-device AllGather of each layer's per-node feature table; incident edges
are resolved with indirect-gather DMAs.

v2 changes vs baseline:
  - fp16 tables + fp16 weights: halves AllGather/gather/load bytes (dummy
    alpha_src uses -60000, fp16-safe, still exp()->0 after leaky).
  - AllGather outputs in Shared address space (fast HBM-HBM collective path).
  - layer-1 message passing runs each dst block in ONE slot chunk (17 slots
    of fp16 rows fit in SBUF), dropping the multi-chunk accumulation.
  - alpha2 is folded into an extended W2 (extra output columns), so layer-2
    attention logits come straight out of the projection matmuls.
  - the final score only reads x2 at the 128 bond endpoints, so layer-2
    message passing is computed for just those nodes (one 128-row block, all
    cores redundantly) — no x2 AllGather, no per-block layer-2 loop.
"""

import sys

if "/opt/trn_rl_repo" not in sys.path:
    sys.path.insert(0, "/opt/trn_rl_repo")

import numpy as np

import concourse.bacc as bacc
import concourse.mybir as mybir
import concourse.tile as tile
from concourse.bass import IndirectOffsetOnAxis
from concourse.bass_utils import run_bass_kernel_spmd
from concourse.masks import make_identity

F32 = mybir.dt.float32
F16 = mybir.dt.float16
I32 = mybir.dt.int32
AF = mybir.ActivationFunctionType
OP = mybir.AluOpType
AX = mybir.AxisListType

N_NODES, N_EDGES = 8192, 49152
IN_F, HID, H1, H2, OUT_F = 128, 64, 64, 5, 32
N_BONDS = 64
N_CORES = 8
NC_SHARD = N_NODES // N_CORES      # 1024 nodes per core
P = 128
NB = NC_SHARD // P                 # 8 dst blocks per core
F1 = H1 * HID                      # 4096
F2 = H2 * OUT_F                    # 160
T1W = F1 + 2 * H1                  # 4224: [h | alpha_src | alpha_dst]
F2E = F2 + 2 * H2                  # 170: [h2 | as2 | ad2]
T2W = 176                          # padded row width of table 2
NEG = -60000.0                     # fp16-safe mask logit


# ---------------------------------------------------------------- host side
def _prep(inputs):
    """Degree-sorted shard permutation, per-block slot tables, bond slots."""
    edge_index = np.asarray(inputs["edge_index"], np.int64)
    src = np.concatenate([edge_index[0], np.arange(N_NODES, dtype=np.int64)])
    dst = np.concatenate([edge_index[1], np.arange(N_NODES, dtype=np.int64)])
    deg = np.bincount(dst, minlength=N_NODES)          # includes self loop
    newpos = np.empty(N_NODES, np.int64)               # node -> permuted global
    for c in range(N_CORES):
        lo = c * NC_SHARD
        order = np.argsort(-deg[lo:lo + NC_SHARD], kind="stable")
        newpos[lo + order] = lo + np.arange(NC_SHARD)
    degp = np.empty(N_NODES, np.int64)
    degp[newpos] = deg
    kb = np.zeros(NB, np.int64)
    for c in range(N_CORES):
        lo = c * NC_SHARD
        blkmax = degp[lo:lo + NC_SHARD].reshape(NB, P).max(axis=1)
        kb = np.maximum(kb, blkmax)
    ks = tuple(int(v) for v in kb)
    tot = int(sum(ks))
    srcidx = np.full((N_CORES, P, tot), N_NODES, np.int32)
    col0 = np.cumsum([0] + list(ks))[:-1]
    for c in range(N_CORES):
        lo = c * NC_SHARD
        rows = np.arange(NC_SHARD) % P
        cols = col0[np.arange(NC_SHARD) // P]
        srcidx[c, rows, cols] = lo + np.arange(NC_SHARD)
    fill = np.ones(N_NODES, np.int64)
    ps = newpos[src[:N_EDGES]]
    pdst = newpos[dst[:N_EDGES]]
    order = np.argsort(pdst, kind="stable")
    ps, pdst = ps[order], pdst[order]
    for s, d in zip(ps.tolist(), pdst.tolist()):
        c, loc = d // NC_SHARD, d % NC_SHARD
        k = fill[d]
        fill[d] += 1
        srcidx[c, loc % P, col0[loc // P] + k] = s

    # bond-endpoint slot table: partition p handles node bonds[p]
    bonds = np.concatenate([np.asarray(inputs["lefts"], np.int64),
                            np.asarray(inputs["rights"], np.int64)])
    slots = [[] for _ in range(P)]
    for p in range(P):
        slots[p].append(int(newpos[bonds[p]]))         # self loop at slot 0
    e_src, e_dst = edge_index[0], edge_index[1]
    by_dst = {}
    for p in range(P):
        by_dst.setdefault(int(bonds[p]), []).append(p)
    for s, d in zip(e_src.tolist(), e_dst.tolist()):
        for p in by_dst.get(d, ()):
            slots[p].append(int(newpos[s]))
    k2 = max(len(sl) for sl in slots)
    srcidx2 = np.full((P, k2), N_NODES, np.int32)
    for p in range(P):
        srcidx2[p, 0:len(slots[p])] = slots[p]
    # t1 table rows are [1024 shard rows + 1 dummy] per core, concatenated
    # by the AllGather: real id g -> g + g//1024, dummy (N_NODES) -> 1024
    def remap(a):
        a = a.astype(np.int64)
        return np.where(a == N_NODES, NC_SHARD,
                        a + a // NC_SHARD).astype(np.int32)
    # t2 table only holds the nodes srcidx2 references: per core a compact,
    # padded selection of S local rows (+1 dummy row), AllGather-concatenated
    sel_loc = [np.unique(srcidx2[(srcidx2 != N_NODES)
                                 & (srcidx2 // NC_SHARD == c)] % NC_SHARD)
               for c in range(N_CORES)]
    s2 = max(len(v) for v in sel_loc)
    s2 = max(P, ((s2 + P - 1) // P) * P)               # multiple of 128
    selidx = np.zeros((N_CORES, s2, 1), np.int32)      # pad -> row 0
    pos = {}
    for c in range(N_CORES):
        selidx[c, 0:len(sel_loc[c]), 0] = sel_loc[c]
        for i, loc in enumerate(sel_loc[c].tolist()):
            pos[c * NC_SHARD + loc] = c * (s2 + 1) + i
    srcidx2c = np.full((P, k2), N_CORES * (s2 + 1) - 1, np.int64)
    for p in range(P):
        for k in range(k2):
            g = int(srcidx2[p, k])
            srcidx2c[p, k] = s2 if g == N_NODES else pos[g]
    return ks, (k2, s2), remap(srcidx), srcidx2c.astype(np.int32), \
        selidx, newpos


def _make_core_inputs(inputs, prep, c):
    ks, _k2s2, srcidx, srcidx2, selidx, newpos = prep
    x = np.asarray(inputs["x"], np.float32)
    W1 = np.asarray(inputs["W1"], np.float32)
    W2 = np.asarray(inputs["W2"], np.float32)
    as2 = np.asarray(inputs["att_src2"], np.float32)
    ad2 = np.asarray(inputs["att_dst2"], np.float32)
    lo = c * NC_SHARD
    sel = np.empty(NC_SHARD, np.int64)                 # permuted pos -> node
    sel[newpos[lo:lo + NC_SHARD] - lo] = np.arange(lo, lo + NC_SHARD)
    # extended W2: rows [W2 | A2s | A2d], A2s[h] = att_src2[h] @ W2_head_h
    a2s = np.einsum("hc,hci->hi", as2, W2.reshape(H2, OUT_F, F1))
    a2d = np.einsum("hc,hci->hi", ad2, W2.reshape(H2, OUT_F, F1))
    W2e = np.concatenate([W2, a2s, a2d], axis=0)       # [170, 4096]
    w2pack = np.zeros((P, (F1 // P) * T2W), np.float16)
    for k in range(F1 // P):
        w2pack[:, k * T2W:k * T2W + F2E] = W2e[:, k * P:(k + 1) * P].T
    dum1 = np.concatenate([np.zeros(F1, np.float32),
                           np.full(H1, NEG, np.float32),
                           np.zeros(H1, np.float32)])
    dum2 = np.concatenate([np.zeros(F2, np.float32),
                           np.full(H2, NEG, np.float32),
                           np.zeros(T2W - F2 - H2, np.float32)])
    return {
        "xT": np.ascontiguousarray(x[sel].T).astype(np.float16),
        "W1s": np.ascontiguousarray(
            W1.T.astype(np.float16)[c * (P // N_CORES):
                                    (c + 1) * (P // N_CORES)]),
        "AsdT": np.concatenate(
            [np.einsum("hc,hci->hi", np.asarray(inputs["att_src1"],
                                                np.float32),
                       W1.reshape(H1, HID, IN_F)).T,
             np.einsum("hc,hci->hi", np.asarray(inputs["att_dst1"],
                                                np.float32),
                       W1.reshape(H1, HID, IN_F)).T],
            axis=1).astype(np.float16),
        "W2es": np.ascontiguousarray(
            w2pack[c * (P // N_CORES):(c + 1) * (P // N_CORES)]),
        "srcidx": np.ascontiguousarray(srcidx[c]),
        "srcidx2": np.ascontiguousarray(srcidx2),
        "selidx": np.ascontiguousarray(selidx[c]),
        "dum1": dum1.astype(np.float16).reshape(P, T1W // P),
        "dum2": dum2.astype(np.float16).reshape(1, T2W),
    }


# ------------------------------------------------------------- device side
def _build_program(key, reps: int = 1, skip=()) -> bacc.Bacc:
    ks, (k2, S) = key
    NB2 = S // P
    nc = bacc.Bacc("TRN2", target_bir_lowering=False, debug=False,
                   num_devices=N_CORES)
    tot = int(sum(ks))
    col0 = np.cumsum([0] + list(ks))[:-1]
    kmax = max(ks)

    xT = nc.dram_tensor("xT", [P, NC_SHARD], F16, kind="ExternalInput")
    W1s = nc.dram_tensor("W1s", [P // N_CORES, F1], F16,
                         kind="ExternalInput")
    AsdT = nc.dram_tensor("AsdT", [P, 2 * H1], F16, kind="ExternalInput")
    W2es = nc.dram_tensor("W2es", [P // N_CORES, (F1 // P) * T2W], F16,
                          kind="ExternalInput")
    srcidx = nc.dram_tensor("srcidx", [P, tot], I32, kind="ExternalInput")
    srcidx2 = nc.dram_tensor("srcidx2", [P, k2], I32, kind="ExternalInput")
    selidx = nc.dram_tensor("selidx", [S, 1], I32, kind="ExternalInput")
    dum1 = nc.dram_tensor("dum1", [P, T1W // P], F16, kind="ExternalInput")
    dum2 = nc.dram_tensor("dum2", [1, T2W], F16, kind="ExternalInput")
    y = nc.dram_tensor("y", [N_BONDS], F32, kind="ExternalOutput")

    rg = [list(range(N_CORES))]
    NK = F1 // P  # 32 k-chunks for the layer-2 projection

    with tile.TileContext(nc, num_cores=N_CORES) as tc, \
            nc.allow_low_precision(reason="fp16 tables; 2e-2 tolerance"):
        with (
            tc.tile_pool(name="dram", bufs=1, space="DRAM") as dpool,
            tc.tile_pool(name="consts", bufs=1) as cpool,
            tc.tile_pool(name="small", bufs=6) as mpool,
            tc.tile_pool(name="ps", bufs=1, space="PSUM") as ppool,
        ):
            srcidx_s = cpool.tile([P, tot], I32)
            srcidx2_s = cpool.tile([P, k2], I32)
            selidx_s = cpool.tile([S, 1], I32)
            dum1_s = cpool.tile([P, T1W // P], F16)
            dum2_s = cpool.tile([1, T2W], F16)
            ident_s = cpool.tile([P, P], F32)
            ident16_s = cpool.tile([P, P], F16)
            for dt_, st_ in [(srcidx_s, srcidx), (srcidx2_s, srcidx2),
                             (selidx_s, selidx),
                             (dum1_s, dum1), (dum2_s, dum2)]:
                nc.sync.dma_start(dt_[:], st_[:])
            make_identity(nc, ident_s[:])
            make_identity(nc, ident16_s[:])
            # weights arrive as per-core row slices; AllGather them once
            w1_full = dpool.tile([P, F1], F16, tag="w1f")
            w2_full = dpool.tile([P, NK * T2W], F16, tag="w2f")
            w1loc = dpool.tile([P // N_CORES, F1], F16, tag="w1l")
            w2loc = dpool.tile([P // N_CORES, NK * T2W], F16, tag="w2l")
            nc.sync.dma_start(w1loc[:], W1s[:])
            nc.sync.dma_start(w2loc[:], W2es[:])
            nc.gpsimd.collective_compute(
                "AllGather", OP.bypass, ins=[w1loc.opt()],
                outs=[w1_full.opt()], replica_groups=rg)
            nc.gpsimd.collective_compute(
                "AllGather", OP.bypass, ins=[w2loc.opt()],
                outs=[w2_full.opt()], replica_groups=rg)

            for _rep in range(reps):
                t1_loc = dpool.tile([NC_SHARD + 1, T1W], F16, tag="t1l")
                t1_full = dpool.tile([N_NODES + N_CORES, T1W], F16,
                                     addr_space="Shared", tag="t1f")
                x1_bf = dpool.tile([NC_SHARD, F1], F16, tag="x1b")
                t2_loc = dpool.tile([S + 1, T2W], F16, tag="t2l")
                t2_full = dpool.tile([(S + 1) * N_CORES, T2W], F16,
                                     addr_space="Shared", tag="t2f")
                x1s_d = dpool.tile([S, F1], F16, tag="x1s")

                # ---- phase A: h1 = x @ W1.T, alpha_s/alpha_d -> table 1
                with (
                    tc.tile_pool(name="pha", bufs=1) as hpool,
                    tc.tile_pool(name="phaw", bufs=2) as wpool,
                ):
                    xT_s = hpool.tile([P, NC_SHARD], F16)
                    w1t_s = hpool.tile([P, F1], F16)
                    asd_s = hpool.tile([P, 2 * H1], F16)
                    for dt_, st_ in [(xT_s, xT), (w1t_s, w1_full),
                                     (asd_s, AsdT)]:
                        nc.sync.dma_start(dt_[:], st_[:])
                    for nt in range(NB if "phA" not in skip else 0):
                        h_s = wpool.tile([P, T1W], F16, tag="h")
                        for hf in range(2):
                            ps_h = ppool.tile([P, F1 // 2], F32, tag="ps")
                            for j in range(4):
                                nc.tensor.matmul(
                                    ps_h[:, j * 512:(j + 1) * 512],
                                    lhsT=xT_s[:, nt * P:(nt + 1) * P],
                                    rhs=w1t_s[:, hf * 2048 + j * 512:
                                              hf * 2048 + (j + 1) * 512],
                                    start=True, stop=True)
                            nc.vector.tensor_copy(
                                h_s[:, hf * 2048:(hf + 1) * 2048], ps_h[:])
                        ps_al = ppool.tile([P, 2 * H1], F32, tag="ps")
                        nc.tensor.matmul(
                            ps_al[:], lhsT=xT_s[:, nt * P:(nt + 1) * P],
                            rhs=asd_s[:], start=True, stop=True)
                        nc.vector.tensor_copy(h_s[:, F1:T1W], ps_al[:])
                        nc.sync.dma_start(
                            t1_loc[nt * P:(nt + 1) * P, :], h_s[:])

                nc.sync.dma_start(
                    t1_loc[NC_SHARD:NC_SHARD + 1, :]
                    .rearrange("o (p j) -> o p j", j=T1W // P).squeeze(0),
                    dum1_s[:])
                if "coll" not in skip:
                    nc.gpsimd.collective_compute(
                        "AllGather", OP.bypass, ins=[t1_loc.opt()],
                        outs=[t1_full.opt()], replica_groups=rg)

                # ---- layer-1 message passing (dst-major, one chunk/block)
                with (
                    tc.tile_pool(name="l1g", bufs=1) as gpool,
                    tc.tile_pool(name="l1w", bufs=2) as wpool,
                ):
                    for blk in range(NB if "l1mp" not in skip else 0):
                        kb = ks[blk]
                        c0 = int(col0[blk])
                        g = gpool.tile([P, kmax * T1W], F16, tag="g")
                        gv = g[:, 0:kb * T1W].rearrange(
                            "p (k w) -> p k w", w=T1W)
                        for k in range(kb):
                            if "gather" in skip:
                                nc.sync.dma_start(
                                    gv[:, k, :], t1_full[0:P, :])
                            else:
                                nc.gpsimd.indirect_dma_start(
                                    out=gv[:, k, :], out_offset=None,
                                    in_=t1_full[:],
                                    in_offset=IndirectOffsetOnAxis(
                                        ap=srcidx_s[:, c0 + k:c0 + k + 1],
                                        axis=0))
                        w = mpool.tile([P, kmax * H1], F16, tag="wsl")
                        wv = w[:, 0:kb * H1].rearrange(
                            "p (k h) -> p k h", h=H1)
                        # logits = leaky(a_src + a_dst) then exp
                        nc.vector.tensor_tensor(
                            out=wv, in0=gv[:, :, F1:F1 + H1],
                            in1=gv[:, 0, F1 + H1:T1W].unsqueeze(1)
                                .broadcast_to([P, kb, H1]),
                            op=OP.add)
                        nc.vector.scalar_tensor_tensor(
                            out=wv, in0=wv, scalar=0.2, in1=wv,
                            op0=OP.mult, op1=OP.max)
                        nc.scalar.activation(wv, wv, AF.Exp)
                        # messages in place, then slot-reduce
                        nc.vector.tensor_tensor(
                            out=gv[:, :, 0:F1]
                                .rearrange("p k (h c) -> p k h c", c=HID),
                            in0=gv[:, :, 0:F1]
                                .rearrange("p k (h c) -> p k h c", c=HID),
                            in1=wv.unsqueeze(3)
                                .broadcast_to([P, kb, H1, HID]),
                            op=OP.mult)
                        num = wpool.tile([P, F1], F32, tag="num", bufs=1)
                        nc.vector.tensor_reduce(
                            out=num[:],
                            in_=gv[:, :, 0:F1].transpose([0, 2, 1]),
                            axis=AX.X, op=OP.add)
                        den = mpool.tile([P, H1], F32, tag="small")
                        nc.vector.tensor_reduce(
                            out=den[:], in_=wv.transpose([0, 2, 1]),
                            axis=AX.X, op=OP.add)
                        # x1 = elu(num / den)   (b1 is zeros by problem spec)
                        dinv = mpool.tile([P, H1], F32, tag="small")
                        nc.vector.reciprocal(dinv[:], den[:])
                        z = wpool.tile([P, F1], F16, tag="z", bufs=1)
                        nc.vector.tensor_tensor(
                            out=z[:].rearrange("p (h c) -> p h c", c=HID),
                            in0=num[:].rearrange("p (h c) -> p h c", c=HID),
                            in1=dinv[:].unsqueeze(2)
                                .broadcast_to([P, H1, HID]),
                            op=OP.mult)
                        zm = wpool.tile([P, F1], F16, tag="zm", bufs=1)
                        nc.vector.tensor_scalar_min(zm[:], z[:], 0.0)
                        nc.scalar.activation(zm[:], zm[:], AF.Exp)
                        nc.scalar.activation(z[:], z[:], AF.Relu)
                        x1n = wpool.tile([P, F1], F16, tag="num", bufs=1)
                        nc.vector.scalar_tensor_tensor(
                            out=x1n[:], in0=zm[:], scalar=-1.0, in1=z[:],
                            op0=OP.add, op1=OP.add)
                        nc.sync.dma_start(
                            x1_bf[blk * P:(blk + 1) * P, :], x1n[:])

                # ---- layer-2 projection z2 = x1 @ W2e.T (feature-major)
                with (
                    tc.tile_pool(name="prj", bufs=2) as jpool,
                    tc.tile_pool(name="prjc", bufs=1) as jc,
                ):
                    w2t_s = jc.tile([P, NK * T2W], F16)
                    nc.sync.dma_start(w2t_s[:], w2_full[:])
                    # stage only the bond-relevant x1 rows (compact S rows)
                    for b2 in range(NB2 if "l2proj" not in skip else 0):
                        xsel = jpool.tile([P, F1], F16, tag="xsel")
                        nc.gpsimd.indirect_dma_start(
                            out=xsel[:], out_offset=None, in_=x1_bf[:],
                            in_offset=IndirectOffsetOnAxis(
                                ap=selidx_s[b2 * P:(b2 + 1) * P, 0:1],
                                axis=0))
                        nc.sync.dma_start(
                            x1s_d[b2 * P:(b2 + 1) * P, :], xsel[:])
                    NN = (S + 511) // 512
                    ps_hi = ppool.tile([P, S], F32, tag="ps")
                    ps_lo = ppool.tile([64, S], F32, tag="ps2")
                    for k in range(NK if "l2proj" not in skip else 0):
                        x1t = jpool.tile([P, S], F16, tag="x1t")
                        nc.sync.dma_start(
                            x1t[:], x1s_d[:, k * P:(k + 1) * P],
                            transpose=True)
                        for nh in range(NN):
                            n0, n1 = nh * 512, min((nh + 1) * 512, S)
                            nc.tensor.matmul(
                                ps_hi[:, n0:n1],
                                lhsT=w2t_s[:, k * T2W:k * T2W + P],
                                rhs=x1t[:, n0:n1],
                                start=(k == 0), stop=(k == NK - 1),
                                skip_group_check=True)
                            nc.tensor.matmul(
                                ps_lo[0:F2E - P, n0:n1],
                                lhsT=w2t_s[:, k * T2W + P:k * T2W + F2E],
                                rhs=x1t[:, n0:n1],
                                start=(k == 0), stop=(k == NK - 1),
                                skip_group_check=True)
                    h2hi = jc.tile([P, S], F16)
                    h2lo = jc.tile([64, S], F16)
                    if "l2proj" not in skip:
                        nc.vector.tensor_copy(h2hi[:], ps_hi[:])
                        nc.vector.tensor_copy(
                            h2lo[0:F2E - P, :], ps_lo[0:F2E - P, :])
                    # transpose back to node-major rows and write table 2
                    for nt in range(NB2 if "l2proj" not in skip else 0):
                        ps_t = ppool.tile([P, T2W], F16, tag="ps2")
                        nc.tensor.transpose(
                            out=ps_t[:, 0:P],
                            in_=h2hi[:, nt * P:(nt + 1) * P],
                            identity=ident16_s[:])
                        nc.tensor.transpose(
                            out=ps_t[:, P:F2E],
                            in_=h2lo[0:F2E - P, nt * P:(nt + 1) * P],
                            identity=ident16_s[0:F2E - P, 0:F2E - P])
                        row = jpool.tile([P, T2W], F16, tag="row")
                        nc.vector.tensor_copy(
                            row[:, 0:F2E], ps_t[:, 0:F2E])
                        nc.sync.dma_start(
                            t2_loc[nt * P:(nt + 1) * P, 0:F2E],
                            row[:, 0:F2E])

                nc.sync.dma_start(t2_loc[S:S + 1, :], dum2_s[:])
                if "coll" not in skip:
                    nc.gpsimd.collective_compute(
                        "AllGather", OP.bypass, ins=[t2_loc.opt()],
                        outs=[t2_full.opt()], replica_groups=rg)

                # ---- layer-2 message passing: only the 128 bond endpoints
                if "l2mp" not in skip:
                    with tc.tile_pool(name="l2g", bufs=1) as g2pool:
                        g2 = g2pool.tile([P, k2 * T2W], F16, tag="g2")
                        gv2 = g2[:].rearrange("p (k w) -> p k w", w=T2W)
                        for k in range(k2):
                            if "gather" in skip:
                                nc.sync.dma_start(
                                    gv2[:, k, :], t2_full[0:P, :])
                            else:
                                nc.gpsimd.indirect_dma_start(
                                    out=gv2[:, k, :], out_offset=None,
                                    in_=t2_full[:],
                                    in_offset=IndirectOffsetOnAxis(
                                        ap=srcidx2_s[:, k:k + 1],
                                        axis=0))
                        w2 = mpool.tile([P, k2 * H2], F32, tag="w2s")
                        wv2 = w2[:, 0:k2 * H2].rearrange(
                            "p (k h) -> p k h", h=H2)
                        nc.vector.tensor_tensor(
                            out=wv2, in0=gv2[:, :, F2:F2 + H2],
                            in1=gv2[:, 0, F2 + H2:F2E].unsqueeze(1)
                                .broadcast_to([P, k2, H2]),
                            op=OP.add)
                        nc.vector.scalar_tensor_tensor(
                            out=wv2, in0=wv2, scalar=0.2, in1=wv2,
                            op0=OP.mult, op1=OP.max)
                        nc.scalar.activation(wv2, wv2, AF.Exp)
                        nc.vector.tensor_tensor(
                            out=gv2[:, :, 0:F2]
                                .rearrange("p k (h c) -> p k h c", c=OUT_F),
                            in0=gv2[:, :, 0:F2]
                                .rearrange("p k (h c) -> p k h c", c=OUT_F),
                            in1=wv2.unsqueeze(3)
                                .broadcast_to([P, k2, H2, OUT_F]),
                            op=OP.mult)
                        num2 = mpool.tile([P, F2], F32, tag="num2")
                        nc.vector.tensor_reduce(
                            out=num2[:],
                            in_=gv2[:, :, 0:F2].transpose([0, 2, 1]),
                            axis=AX.X, op=OP.add)
                        den2 = mpool.tile([P, H2], F32, tag="small")
                        nc.vector.tensor_reduce(
                            out=den2[:], in_=wv2.transpose([0, 2, 1]),
                            axis=AX.X, op=OP.add)
                        dinv2 = mpool.tile([P, H2], F32, tag="small")
                        nc.vector.reciprocal(dinv2[:], den2[:])
                        nc.vector.tensor_scalar_mul(
                            dinv2[:], dinv2[:], 1.0 / H2)   # fold head-mean
                        nc.vector.tensor_tensor(
                            out=num2[:].rearrange(
                                "p (h c) -> p h c", c=OUT_F),
                            in0=num2[:].rearrange(
                                "p (h c) -> p h c", c=OUT_F),
                            in1=dinv2[:].unsqueeze(2)
                                .broadcast_to([P, H2, OUT_F]),
                            op=OP.mult)
                        # x2[p] summed over features -> u[p]; then bond
                        # score s[b] = u[b] + u[64+b]; softmax over bonds
                        u = mpool.tile([P, 1], F32, tag="small")
                        nc.vector.tensor_reduce(
                            out=u[:], in_=num2[:], axis=AX.X, op=OP.add)
                        ps_u = ppool.tile([1, P], F32, tag="ps2")
                        nc.tensor.transpose(
                            out=ps_u[:], in_=u[:], identity=ident_s[:])
                        u_row = mpool.tile([1, P], F32, tag="small")
                        nc.vector.tensor_copy(u_row[:], ps_u[:])
                        s = mpool.tile([1, N_BONDS], F32, tag="small")
                        nc.vector.tensor_tensor(
                            out=s[:], in0=u_row[0:1, 0:N_BONDS],
                            in1=u_row[0:1, N_BONDS:2 * N_BONDS], op=OP.add)
                        nc.scalar.activation(s[:], s[:], AF.Exp)
                        ssum = mpool.tile([1, 1], F32, tag="small")
                        nc.vector.tensor_reduce(
                            out=ssum[:], in_=s[:], axis=AX.X, op=OP.add)
                        sinv = mpool.tile([1, 1], F32, tag="small")
                        nc.vector.reciprocal(sinv[:], ssum[:])
                        ys = mpool.tile([1, N_BONDS], F32, tag="small")
                        nc.vector.tensor_tensor(
                            out=ys[:], in0=s[:],
                            in1=sinv[:].to_broadcast([1, N_BONDS]),
                            op=OP.mult)
                        nc.sync.dma_start(y.ap().unsqueeze(0), ys[:])

    nc.compile()
    return nc


_PROGRAM_CACHE: dict = {}


def kernel(**inputs) -> np.ndarray:
    prep = _prep(inputs)
    key = (prep[0], prep[1])
    if key not in _PROGRAM_CACHE:
        _PROGRAM_CACHE[key] = _build_program(key)
    nc = _PROGRAM_CACHE[key]
    in_maps = [_make_core_inputs(inputs, prep, c) for c in range(N_CORES)]
    res = run_bass_kernel_spmd(nc, in_maps, core_ids=list(range(N_CORES)))
    return res.results[0]["y"]


if __name__ == "__main__":
    import jax

    import reference

    with jax.default_device(jax.devices("cpu")[0]):
        inputs = {k: np.asarray(v) for k, v in reference.setup_inputs().items()}
        expected = np.asarray(reference.reference(**reference.setup_inputs()))
    actual = kernel(**inputs)
    rel = np.abs(actual - expected).max() / np.abs(expected).max()
    print("Relative error:", rel)
